# revision 14
# baseline (speedup 1.0000x reference)
"""Chunkwise causal linear attention, sequence-parallel on 8 TRN2 NeuronCores.

Sharding: core r = 2*b + h owns batch b, sequence half h (2048 tokens,
8 chunks of L=256). Phase A computes per-chunk KV states and their
running prefix (the chain), plus k^T via PE transpose; a pair AllGather
ships each first-half core's total KV state to its second-half partner;
phase B1 computes q and masked scores for every chunk (overlapping the
collective); phase B2 adds the inter-chunk term q @ S and applies the
output projection. The math is the reference's chunked linear attention
re-chunked at L=256 (identical up to fp reassociation); compute runs in
bf16 with fp32 PSUM accumulation.
"""
import sys
import types

sys.path.insert(0, "/opt/trn_rl_repo")

import numpy as np

P = 128
D = 512
L = 256          # chunk size on device
NCH = 8          # chunks per core
NTOK = L * NCH   # tokens per core
DT = D // P      # 4
T2 = L // P      # 2
B, T = 4, 4096
N_CORES = 8

_CACHE = {}


def _ensure_axon_hooks():
    """antenv.axon_hooks is missing from this image; recreate it so
    run_bass_kernel_spmd(trace=True) can NTFF-profile via libaxon."""
    if "antenv.axon_hooks" in sys.modules:
        return
    try:
        import antenv  # noqa: F401
    except ImportError:
        return
    mod = types.ModuleType("antenv.axon_hooks")
    hook = [None]
    mod.set_axon_ntff_profile_hook = lambda h: hook.__setitem__(0, h)
    mod.get_axon_ntff_profile_hook = lambda: hook[0]
    sys.modules["antenv.axon_hooks"] = mod
    try:
        from trn_agent_boot.trn_boot import _ntff_profile_via_ctypes

        h = _ntff_profile_via_ctypes("/opt/axon/libaxon_pjrt.so")
        if h is not None:
            mod.set_axon_ntff_profile_hook(h)
    except Exception:
        pass


def _build():
    import concourse.bass as bass  # noqa: F401
    import concourse.mybir as mybir
    import concourse.tile as tile
    from concourse import bacc
    from concourse.bass_interp import get_hw_module
    from concourse.masks import make_identity

    f32 = mybir.dt.float32
    bf16 = mybir.dt.bfloat16

    nc = bacc.Bacc("TRN2", target_bir_lowering=False, debug=False,
                   enable_asserts=True, num_devices=N_CORES)

    xT_d = nc.dram_tensor("xT", [D, NTOK], bf16, kind="ExternalInput")
    wq_d = nc.dram_tensor("wqT", [D, D], bf16, kind="ExternalInput")
    wk_d = nc.dram_tensor("wkT", [D, D], bf16, kind="ExternalInput")
    wv_d = nc.dram_tensor("wvT", [D, D], bf16, kind="ExternalInput")
    wo_d = nc.dram_tensor("woT", [D, D], bf16, kind="ExternalInput")
    mask_d = nc.dram_tensor("mask", [P, T2, L], bf16, kind="ExternalInput")
    hsel_d = nc.dram_tensor("hsel", [P, 1], f32, kind="ExternalInput")
    y_d = nc.dram_tensor("y", [NTOK, D], f32, kind="ExternalOutput")

    xT_ap = xT_d.ap().rearrange("(a p) t -> p a t", p=P)

    with tile.TileContext(nc) as tc:
        with tc.tile_pool(name="const", bufs=1) as constp, \
             tc.tile_pool(name="store", bufs=1) as storep, \
             tc.tile_pool(name="work", bufs=3) as workp, \
             tc.tile_pool(name="dram", bufs=1, space="DRAM") as dramp:

            # --- resident constants & x ---------------------------------
            wq_s = constp.tile([P, DT, D], bf16)
            wk_s = constp.tile([P, DT, D], bf16)
            wv_s = constp.tile([P, DT, D], bf16)
            wo_s = constp.tile([P, DT, D], bf16)
            x_s = constp.tile([P, DT, NTOK], bf16)
            ident = constp.tile([P, P], bf16)
            make_identity(nc, ident[:])
            # phase A needs wk + x chunk 0 + wv first, on the fast queue;
            # split into per-a slices so the first matmuls start early
            wk_ap = wk_d.ap().rearrange("(a p) n -> p a n", p=P)
            for a in range(DT):
                nc.sync.dma_start(wk_s[:, a], wk_ap[:, a])
                nc.sync.dma_start(x_s[:, a, 0:L], xT_ap[:, a, 0:L])
            nc.sync.dma_start(wv_s[:], wv_d.ap().rearrange(
                "(a p) n -> p a n", p=P))
            for c in range(1, NCH):
                nc.sync.dma_start(x_s[:, :, c * L:(c + 1) * L],
                                  xT_ap[:, :, c * L:(c + 1) * L])
            nc.gpsimd.dma_start(wq_s[:], wq_d.ap().rearrange(
                "(a p) n -> p a n", p=P))
            nc.gpsimd.dma_start(wo_s[:], wo_d.ap().rearrange(
                "(a p) n -> p a n", p=P))
            mask_s = constp.tile([P, T2, L], bf16)
            nc.gpsimd.dma_start(mask_s[:], mask_d.ap())
            hsel_s = constp.tile([P, 1], f32)
            nc.gpsimd.dma_start(hsel_s[:], hsel_d.ap())

            # --- persistent state ---------------------------------------
            v_all = storep.tile([P, NCH, T2, D], bf16)
            kf_all = storep.tile([P, NCH, DT, L], bf16)
            chain = storep.tile([P, NCH, DT, D], bf16)
            q_all = storep.tile([P, DT, NTOK], bf16)
            sc_all = storep.tile([P, NCH, T2, L], bf16)
            ai_all = storep.tile([P, NCH, DT, L], bf16)
            S_base = storep.tile([P, DT, D], bf16)

            cc_in = dramp.tile([P, DT, D], bf16)
            cc_out = dramp.tile([2, P, DT, D], bf16)

            def xc(c):
                return x_s[:, :, c * L:(c + 1) * L]

            # --- HAM pre-warm: ~3.4us of dummy PE activity on the identity
            # while the first weight/x DMAs are still in flight, so real
            # matmuls start at 2.4GHz instead of the cold 1.2GHz ------------
            with tc.tile_pool(name="ppW", bufs=1, space="PSUM") as ppW:
                wps = ppW.tile([P, P], f32)
                for i in range(64):
                    nc.tensor.matmul(wps[:], ident[:], ident[:],
                                     start=(i == 0), stop=(i == 63))

            # --- phase A: k,v projections, kv chain, k transpose --------
            with tc.tile_pool(name="ppA", bufs=3, space="PSUM") as ppA, \
                 tc.tile_pool(name="ppKV", bufs=3, space="PSUM") as ppKV, \
                 tc.tile_pool(name="ppTr", bufs=2, space="PSUM") as ppTr:
                for c in range(NCH):
                    k_T = workp.tile([P, T2, D], bf16, tag="kT")
                    for t2 in range(T2):
                        ps = ppA.tile([P, D], f32, tag="projA")
                        for a in range(DT):
                            nc.tensor.matmul(
                                ps[:], xc(c)[:, a, t2 * P:(t2 + 1) * P],
                                wk_s[:, a, :],
                                start=(a == 0), stop=(a == DT - 1))
                        nc.scalar.copy(out=k_T[:, t2, :], in_=ps[:])
                    for t2 in range(T2):
                        ps = ppA.tile([P, D], f32, tag="projA")
                        for a in range(DT):
                            nc.tensor.matmul(
                                ps[:], xc(c)[:, a, t2 * P:(t2 + 1) * P],
                                wv_s[:, a, :],
                                start=(a == 0), stop=(a == DT - 1))
                        nc.scalar.copy(out=v_all[:, c, t2, :], in_=ps[:])

                    for i2 in range(DT):
                        ps = ppKV.tile([P, D], f32, tag="kvps")
                        for t2 in range(T2):
                            nc.tensor.matmul(
                                ps[:], k_T[:, t2, i2 * P:(i2 + 1) * P],
                                v_all[:, c, t2, :],
                                start=(t2 == 0), stop=(t2 == T2 - 1))
                        if c == 0:
                            nc.vector.tensor_copy(
                                out=chain[:, 0, i2, :], in_=ps[:])
                        else:
                            nc.vector.tensor_add(
                                out=chain[:, c, i2, :],
                                in0=chain[:, c - 1, i2, :], in1=ps[:])

                    # k_F blocks via PE transpose of k_T
                    for i2 in range(DT):
                        for t2 in range(T2):
                            tps = ppTr.tile([P, P], bf16, tag="trps")
                            nc.tensor.transpose(
                                tps[:], k_T[:, t2, i2 * P:(i2 + 1) * P],
                                ident[:])
                            nc.vector.tensor_copy(
                                out=kf_all[:, c, i2, t2 * P:(t2 + 1) * P],
                                in_=tps[:])

            # --- collective: pair AllGather of the local KV total -------
            nc.sync.dma_start(cc_in[:], chain[:, NCH - 1])
            nc.gpsimd.collective_compute(
                "AllGather", mybir.AluOpType.bypass,
                replica_groups=[[0, 1], [2, 3], [4, 5], [6, 7]],
                ins=[cc_in.opt()], outs=[cc_out.opt()],
            )

            # --- phase B1 (collective-independent): q, scores -----------
            with tc.tile_pool(name="ppP", bufs=2, space="PSUM") as ppP, \
                 tc.tile_pool(name="ppS", bufs=2, space="PSUM") as ppS:
                for g in range(NCH // 2):
                    for n2 in range(DT):
                        ps = ppP.tile([P, D], f32, tag="projps")
                        for a in range(DT):
                            nc.tensor.matmul(
                                ps[:], wq_s[:, a, n2 * P:(n2 + 1) * P],
                                x_s[:, a, g * 2 * L:(g + 1) * 2 * L],
                                start=(a == 0), stop=(a == DT - 1))
                        nc.scalar.copy(
                            out=q_all[:, n2, g * 2 * L:(g + 1) * 2 * L],
                            in_=ps[:])
                for c in range(NCH):
                    nc.gpsimd.memset(sc_all[:, c, 1, 0:P], 0.0)
                    for s2 in range(T2):
                        lo = s2 * P  # tokens t < s2*P are fully masked
                        ps = ppS.tile([P, L], f32, tag="scps")
                        for a in range(DT):
                            nc.tensor.matmul(
                                ps[:, lo:], kf_all[:, c, a, s2 * P:(s2 + 1) * P],
                                q_all[:, a, c * L + lo:(c + 1) * L],
                                start=(a == 0), stop=(a == DT - 1))
                        nc.vector.tensor_mul(
                            out=sc_all[:, c, s2, lo:], in0=ps[:, lo:],
                            in1=mask_s[:, s2, lo:])
                    for j2 in range(DT):
                        ps = ppS.tile([P, L], f32, tag="iaps")
                        for s2 in range(T2):
                            nc.tensor.matmul(
                                ps[:], v_all[:, c, s2, j2 * P:(j2 + 1) * P],
                                sc_all[:, c, s2, :],
                                start=(s2 == 0), stop=(s2 == T2 - 1))
                        if j2 % 2 == 0:
                            nc.scalar.copy(out=ai_all[:, c, j2, :], in_=ps[:])
                        else:
                            nc.vector.tensor_copy(out=ai_all[:, c, j2, :],
                                                  in_=ps[:])

            # --- collective landing: S_base = hsel * partner total ------
            G0_s = workp.tile([P, DT, D], bf16, tag="G0")
            for i2 in range(DT):
                nc.gpsimd.dma_start(G0_s[:, i2], cc_out[0, :, i2])
                nc.vector.tensor_scalar_mul(
                    S_base[:, i2, :], G0_s[:, i2, :], hsel_s[:, :])

            # --- phase B2: intra + inter attention, output projection ---
            with tc.tile_pool(name="ppAt", bufs=4, space="PSUM") as ppAt, \
                 tc.tile_pool(name="ppY", bufs=4, space="PSUM") as ppY:
                for c in range(NCH):
                    if c == 0:
                        S_tot = S_base
                    else:
                        S_tot = workp.tile([P, DT, D], bf16, tag="Stot")
                        for i2 in range(DT):
                            nc.vector.tensor_add(
                                out=S_tot[:, i2, :], in0=S_base[:, i2, :],
                                in1=chain[:, c - 1, i2, :])

                    attn = workp.tile([P, DT, L], bf16, tag="attn")
                    for j2 in range(DT):
                        ps = ppAt.tile([P, L], f32, tag="atps")
                        for i2 in range(DT):
                            nc.tensor.matmul(
                                ps[:], S_tot[:, i2, j2 * P:(j2 + 1) * P],
                                q_all[:, i2, c * L:(c + 1) * L],
                                start=(i2 == 0), stop=(i2 == DT - 1))
                        nc.vector.tensor_add(out=attn[:, j2, :],
                                             in0=ai_all[:, c, j2, :],
                                             in1=ps[:])

                    for t2 in range(T2):
                        ps = ppY.tile([P, D], f32, tag="yps")
                        for j2 in range(DT):
                            nc.tensor.matmul(
                                ps[:], attn[:, j2, t2 * P:(t2 + 1) * P],
                                wo_s[:, j2, :],
                                start=(j2 == 0), stop=(j2 == DT - 1))
                        y_sb = workp.tile([P, D], f32, tag="ysb")
                        if t2 % 2 == 0:
                            nc.scalar.copy(out=y_sb[:], in_=ps[:])
                        else:
                            nc.vector.tensor_copy(out=y_sb[:], in_=ps[:])
                        row = c * L + t2 * P
                        nc.sync.dma_start(y_d.ap()[row:row + P, :], y_sb[:])

    nc.compile()
    nc.m = get_hw_module(nc.m)
    return nc


def _get_nc():
    if "nc" not in _CACHE:
        _ensure_axon_hooks()
        _CACHE["nc"] = _build()
    return _CACHE["nc"]


def _bf16(a):
    import ml_dtypes
    return np.asarray(a, np.float32).astype(ml_dtypes.bfloat16)


def make_in_maps(x, wq, wk, wv, wo):
    x = np.asarray(x, dtype=np.float32)
    wqT = _bf16(np.asarray(wq, np.float32).T)
    wkT = _bf16(np.asarray(wk, np.float32).T)
    wvT = _bf16(np.asarray(wv, np.float32).T)
    woT = _bf16(np.asarray(wo, np.float32).T)
    s = np.arange(P)[:, None] + np.arange(T2)[None, :] * P  # [P, T2]
    mask = _bf16(s[:, :, None] <= np.arange(L)[None, None, :])
    in_maps = []
    for r in range(N_CORES):
        b, h = r // 2, r % 2
        xT = _bf16(x[b, h * NTOK:(h + 1) * NTOK, :].T)
        in_maps.append({
            "xT": xT, "wqT": wqT, "wkT": wkT, "wvT": wvT, "woT": woT,
            "mask": mask, "hsel": np.full((P, 1), float(h), np.float32),
        })
    return in_maps


def run_spmd(in_maps, trace=False):
    from concourse.bass_utils import run_bass_kernel_spmd

    nc = _get_nc()
    return run_bass_kernel_spmd(nc, in_maps, core_ids=list(range(N_CORES)),
                                trace=trace)


def kernel(x, wq, wk, wv, wo):
    in_maps = make_in_maps(x, wq, wk, wv, wo)
    try:
        res = run_spmd(in_maps)
    except Exception:
        import time

        time.sleep(10)  # transient axon/NRT hiccups recover on retry
        res = run_spmd(in_maps)
    out = np.empty((B, T, D), np.float32)
    for r in range(N_CORES):
        b, h = r // 2, r % 2
        out[b, h * NTOK:(h + 1) * NTOK, :] = res.results[r]["y"]
    return out
